# revision 15
# baseline (speedup 1.0000x reference)
"""MoE FFN (8 experts, top-2) on 8 Trainium2 NeuronCores.

Expert parallelism with NS-way hidden-dim sharding for load balance: the
router runs on host (same jax ops as the reference). Each expert's FFN is
split along the hidden dim into NS parts computed on NS different cores; with
NS=8 every core owns a distinct H/8 slice of EVERY expert, so per-core work
is exactly sum(counts)/8 token-equivalents -- perfect balance regardless of
routing skew. The host sums the NS partial outputs per expert, adds b2,
applies the combine weights, and scatter-adds into the final output.

On-device layout: contraction dim lives on SBUF partitions for every matmul.
Weights are host-prearranged so each consumed [128,128] stationary block
arrives as part of a single [128 x 2KB-line] descriptor in exact consumption
order, spread over the sync/scalar HW DGE queues (startup-critical pieces)
and the software gpsimd queue (everything with slack). PSUM accumulates f32;
the layer-1 bias rides the gelu on ScalarE; layer-2 output is evicted to
bf16 (b2 is added on host). Layer 2's contraction is issued in two halves so
the last-gelu latency hides under the first half's matmuls instead of
stalling the PE at each tile boundary.
"""

import os
import numpy as np
import ml_dtypes

N_EXPERTS = 8
TOP_K = 2
C = 1024
H = 4096
P = 128
T_TILE = 512
KO1 = C // P   # 8 contraction chunks for layer 1

NS = int(os.environ.get("MOE_NSLOT", "8"))   # experts split NS ways
EPS = 8 // NS                                # experts per slot
HP = H // NS                                 # hidden rows per part
MHP = HP // P                                # m-chunks per slot

_nc_cache = {}


def _split_tiles(cap: int, ramp: bool = False):
    # Near-equal token tiles, each <= T_TILE, sized to keep matmuls above the
    # LDWEIGHTS floor. With ramp=True the first tiles are small so the PE
    # starts while the cold DMA queues are still slow.
    if ramp and cap >= 1152:
        return [256, 384] + _split_tiles(cap - 640)
    n = -(-cap // T_TILE)
    return [cap // n + (1 if i < cap % n else 0) for i in range(n)]


def _build_nc(caps: tuple):
    import concourse.mybir as mybir
    import concourse.tile as tile
    from concourse import bacc

    bf16 = mybir.dt.bfloat16
    f32 = mybir.dt.float32
    gelu = mybir.ActivationFunctionType.Gelu_apprx_tanh

    nc = bacc.Bacc()
    dram = {}
    for s, cap in enumerate(caps):
        dram[f"xt{s}"] = nc.dram_tensor(f"xt{s}", [C, cap], bf16, kind="ExternalInput")
        dram[f"w1t{s}"] = nc.dram_tensor(f"w1t{s}", [P, MHP, C], bf16, kind="ExternalInput")
        dram[f"w2t{s}"] = nc.dram_tensor(f"w2t{s}", [P, MHP, C], bf16, kind="ExternalInput")
        dram[f"b1{s}"] = nc.dram_tensor(f"b1{s}", [P, MHP], f32, kind="ExternalInput")
        dram[f"yt{s}"] = nc.dram_tensor(f"yt{s}", [C, cap], bf16, kind="ExternalOutput")

    xr = [dram[f"xt{s}"].rearrange("(ko ki) t -> ki ko t", ki=P) for s in range(NS)]
    yr = [dram[f"yt{s}"].rearrange("(co p) t -> p co t", p=P) for s in range(NS)]

    tiles = [_split_tiles(caps[s], ramp=(s == 0)) for s in range(NS)]
    sched = []
    slot_start = []
    for s in range(NS):
        t0 = 0
        slot_start.append(len(sched))
        for T in tiles[s]:
            sched.append((s, T, t0))
            t0 += T
    # issue slot-k constants two tiles before slot k starts (>=40us of slack)
    const_at = {max(2, slot_start[s] - 2): s for s in range(1, NS)}

    with tile.TileContext(nc) as tc:
        with (
            tc.tile_pool(name="const", bufs=1) as const,
            tc.tile_pool(name="xp", bufs=2) as xp,
            tc.tile_pool(name="gp", bufs=1) as gp,
            tc.tile_pool(name="yp", bufs=6) as yp,
            tc.tile_pool(name="psum", bufs=8, space="PSUM") as psum,
        ):
            w1_sb = [
                const.tile([P, MHP, C], bf16, tag=f"w1{s}", name=f"w1{s}")
                for s in range(NS)
            ]
            w2_sb = [
                const.tile([P, MHP, C], bf16, tag=f"w2{s}", name=f"w2{s}")
                for s in range(NS)
            ]
            b1_sb = [
                const.tile([P, MHP], f32, tag=f"b1{s}", name=f"b1{s}")
                for s in range(NS)
            ]

            def load_consts(s, first_on_scalar=0):
                for m in range(MHP):
                    eng = nc.scalar if m < first_on_scalar else nc.gpsimd
                    eng.dma_start(w1_sb[s][:, m : m + 1, :], dram[f"w1t{s}"][:, m : m + 1, :])
                for ho in range(MHP):
                    nc.gpsimd.dma_start(
                        w2_sb[s][:, ho : ho + 1, :], dram[f"w2t{s}"][:, ho : ho + 1, :]
                    )

            # --- initial loads: x0 on sync; b1+first w1 m-chunks on scalar
            # (fast HW queue); the rest of slot 0 on gpsimd.
            x_tiles = {}
            T0 = tiles[0][0]
            x_tiles[0] = xp.tile([P, KO1, T_TILE], bf16, tag="x", name="x0")
            nc.sync.dma_start(x_tiles[0][:, :, :T0], xr[0][:, :, :T0])
            nc.scalar.dma_start(b1_sb[0][:], dram["b10"][:])
            load_consts(0, first_on_scalar=2)

            for gi, (s, T, t0) in enumerate(sched):
                if gi + 1 < len(sched):
                    ns_, nT, nt0 = sched[gi + 1]
                    x_tiles[gi + 1] = xp.tile(
                        [P, KO1, T_TILE], bf16, tag="x", name=f"x{gi + 1}"
                    )
                    nc.sync.dma_start(
                        x_tiles[gi + 1][:, :, :nT], xr[ns_][:, :, nt0 : nt0 + nT]
                    )
                if gi in const_at:
                    sc = const_at[gi]
                    nc.gpsimd.dma_start(b1_sb[sc][:], dram[f"b1{sc}"][:])
                    load_consts(sc)

                x_sb = x_tiles.pop(gi)
                g_sb = gp.tile([P, MHP, T_TILE], bf16, tag="g", name=f"g{gi}")
                for m in range(MHP):
                    ph = psum.tile([P, T_TILE], f32, tag="ps", name=f"ph{gi}_{m}")
                    for ko in range(KO1):
                        nc.tensor.matmul(
                            ph[:, :T],
                            w1_sb[s][:, m, ko * P : (ko + 1) * P],
                            x_sb[:, ko, :T],
                            start=(ko == 0),
                            stop=(ko == KO1 - 1),
                        )
                    nc.scalar.activation(
                        g_sb[:, m, :T], ph[:, :T], gelu, bias=b1_sb[s][:, m : m + 1]
                    )
                # layer 2, contraction split in two so the first half issues
                # behind the L1 matmuls while the last gelu drains.
                pys = [
                    psum.tile([P, T_TILE], f32, tag="ps", name=f"py{gi}_{co}")
                    for co in range(KO1)
                ]
                for co in range(KO1):
                    for ho in range(MHP // 2):
                        nc.tensor.matmul(
                            pys[co][:, :T],
                            w2_sb[s][:, ho, co * P : (co + 1) * P],
                            g_sb[:, ho, :T],
                            start=(ho == 0),
                            stop=False,
                        )
                for co in range(KO1):
                    for ho in range(MHP // 2, MHP):
                        nc.tensor.matmul(
                            pys[co][:, :T],
                            w2_sb[s][:, ho, co * P : (co + 1) * P],
                            g_sb[:, ho, :T],
                            start=False,
                            stop=(ho == MHP - 1),
                        )
                    y_sb = yp.tile([P, T_TILE], bf16, tag="y", name=f"y{gi}_{co}")
                    nc.vector.tensor_copy(y_sb[:, :T], pys[co][:, :T])
                    nc.sync.dma_start(yr[s][:, co, t0 : t0 + T], y_sb[:, :T])
    nc.finalize()
    return nc


def _route(flat_f32: np.ndarray, gate_w: np.ndarray):
    """Router, bit-matching the reference's jax ops (same env/backend)."""
    import jax
    import jax.numpy as jnp

    logits = jnp.asarray(flat_f32) @ jnp.asarray(gate_w).T
    probs = jax.nn.softmax(logits, axis=-1)
    top_p, top_i = jax.lax.top_k(probs, TOP_K)
    weights = top_p / (jnp.sum(top_p, axis=-1, keepdims=True) + 1e-8)
    return np.asarray(top_i), np.asarray(weights)


# results of the last device run, for test harness introspection
last_result = None


def _ensure_ntff_hook():
    """bass_utils' trace path imports antenv.axon_hooks, which the agent
    image's antenv lacks. Build the hook from trn_agent_boot's ctypes
    shim and inject a stand-in module."""
    import sys
    import types

    if "antenv.axon_hooks" in sys.modules:
        return
    try:
        from trn_agent_boot.trn_boot import _ntff_profile_via_ctypes

        hook = _ntff_profile_via_ctypes("/opt/axon/libaxon_pjrt.so")
    except Exception:
        hook = None
    m = types.ModuleType("antenv.axon_hooks")
    m.get_axon_ntff_profile_hook = lambda: hook
    m.set_axon_ntff_profile_hook = lambda h: None
    sys.modules["antenv.axon_hooks"] = m


def _prep_w1(w1e_part: np.ndarray) -> np.ndarray:
    # [HP, C] -> [ki, m, ko*128+j] holding w1[m*128+j, ko*128+ki]
    bf16 = ml_dtypes.bfloat16
    a = w1e_part.reshape(MHP, P, KO1, P)            # [m, j, ko, ki]
    a = a.transpose(3, 0, 2, 1).reshape(P, MHP, C)  # [ki, m, (ko j)]
    return np.ascontiguousarray(a).astype(bf16)


def _prep_w2(w2e_part: np.ndarray) -> np.ndarray:
    # [C, HP] -> [ki, ho, co*128+j] holding w2[co*128+j, ho*128+ki]
    bf16 = ml_dtypes.bfloat16
    a = w2e_part.reshape(KO1, P, MHP, P)            # [co, j, ho, ki]
    a = a.transpose(3, 2, 0, 1).reshape(P, MHP, C)  # [ki, ho, (co j)]
    return np.ascontiguousarray(a).astype(bf16)


def kernel(x, gate_w, w1, b1, w2, b2):
    from concourse.bass_utils import run_bass_kernel_spmd

    x = np.asarray(x)
    B, N, _ = x.shape
    flat = np.ascontiguousarray(x.reshape(-1, C), dtype=np.float32)
    w1 = np.asarray(w1, dtype=np.float32)
    w2 = np.asarray(w2, dtype=np.float32)
    b1 = np.asarray(b1, dtype=np.float32)
    b2 = np.asarray(b2, dtype=np.float32)

    top_i, weights = _route(flat, np.asarray(gate_w, dtype=np.float32))

    idx_e, g_e = [], []
    for e in range(N_EXPERTS):
        rows, cols = np.nonzero(top_i == e)
        idx_e.append(rows)
        g_e.append(weights[rows, cols].astype(np.float32))
    counts = np.array([len(i) for i in idx_e])

    # rank experts by load. Slot s serves experts ranked[s*EPS:(s+1)*EPS];
    # core c computes hidden-part (c // EPS) of expert ranked[s*EPS + c%EPS].
    ranked = np.argsort(-counts, kind="stable")
    caps = tuple(
        int(max(counts[ranked[s * EPS + j]] for j in range(EPS)))
        for s in range(NS)
    )

    nc = _nc_cache.get(caps)
    if nc is None:
        nc = _build_nc(caps)
        _nc_cache[caps] = nc

    bf16 = ml_dtypes.bfloat16

    # per-expert padded x, shared by all cores serving that expert
    xt = {}
    for s in range(NS):
        for j in range(EPS):
            e = int(ranked[s * EPS + j])
            xe = np.zeros((C, caps[s]), dtype=bf16)
            xe[:, : counts[e]] = flat[idx_e[e]].T.astype(bf16)
            xt[e] = xe

    in_maps = []
    for core in range(8):
        j, hpart = core % EPS, core // EPS
        m = {}
        lo, hi = hpart * HP, (hpart + 1) * HP
        for s in range(NS):
            e = int(ranked[s * EPS + j])
            m[f"xt{s}"] = xt[e]
            m[f"w1t{s}"] = _prep_w1(w1[e, lo:hi, :])
            m[f"w2t{s}"] = _prep_w2(w2[e, :, lo:hi])
            m[f"b1{s}"] = np.ascontiguousarray(b1[e, lo:hi].reshape(MHP, P).T)
        in_maps.append(m)

    trace = bool(int(os.environ.get("MOE_TRACE", "0")))
    if trace:
        _ensure_ntff_hook()

    global last_result
    res = run_bass_kernel_spmd(
        nc,
        in_maps,
        core_ids=list(range(8)),
        trace=trace,
    )
    last_result = res

    out = np.zeros((flat.shape[0], C), dtype=np.float32)
    for s in range(NS):
        for j in range(EPS):
            e = int(ranked[s * EPS + j])
            cnt = counts[e]
            y = np.zeros((C, cnt), dtype=np.float32)
            for hpart in range(NS):
                y += res.results[hpart * EPS + j][f"yt{s}"][:, :cnt].astype(np.float32)
            out[idx_e[e]] += g_e[e][:, None] * (y.T + b2[e])
    return out.reshape(B, N, C)


# revision 18
# speedup vs baseline: 1.0024x; 1.0024x over previous
"""MoE FFN (8 experts, top-2) on 8 Trainium2 NeuronCores.

Expert parallelism with NS-way hidden-dim sharding for load balance: the
router runs on host (same jax ops as the reference). Each expert's FFN is
split along the hidden dim into NS parts computed on NS different cores; with
NS=8 every core owns a distinct H/8 slice of EVERY expert, so per-core work
is exactly sum(counts)/8 token-equivalents -- perfect balance regardless of
routing skew. The host sums the NS partial outputs per expert, adds b2,
applies the combine weights, and scatter-adds into the final output.

On-device layout: contraction dim lives on SBUF partitions for every matmul.
Weights are host-prearranged so each consumed [128,128] stationary block
arrives as part of a single [128 x 2KB-line] descriptor in exact consumption
order, spread over the sync/scalar HW DGE queues (startup-critical pieces)
and the software gpsimd queue (everything with slack). PSUM accumulates f32;
the layer-1 bias rides the gelu on ScalarE; layer-2 output is evicted to
bf16 (b2 is added on host). Layer 2's contraction is issued in two halves so
the last-gelu latency hides under the first half's matmuls instead of
stalling the PE at each tile boundary.
"""

import os
import numpy as np
import ml_dtypes

N_EXPERTS = 8
TOP_K = 2
C = 1024
H = 4096
P = 128
T_TILE = 512
KO1 = C // P   # 8 contraction chunks for layer 1

NS = int(os.environ.get("MOE_NSLOT", "2"))   # experts split NS ways
EPS = 8 // NS                                # experts per slot
HP = H // NS                                 # hidden rows per part
MHP = HP // P                                # m-chunks per slot

_nc_cache = {}


def _split_tiles(cap: int, ramp: bool = False):
    # Near-equal token tiles, each <= T_TILE, sized to keep matmuls above the
    # LDWEIGHTS floor. With ramp=True the first tiles are small so the PE
    # starts while the cold DMA queues are still slow.
    if ramp and cap >= 1152:
        return [256, 384] + _split_tiles(cap - 640)
    n = -(-cap // T_TILE)
    return [cap // n + (1 if i < cap % n else 0) for i in range(n)]


def _build_nc(caps: tuple):
    import concourse.mybir as mybir
    import concourse.tile as tile
    from concourse import bacc

    bf16 = mybir.dt.bfloat16
    f32 = mybir.dt.float32
    gelu = mybir.ActivationFunctionType.Gelu_apprx_tanh

    nc = bacc.Bacc()
    dram = {}
    for s, cap in enumerate(caps):
        dram[f"xt{s}"] = nc.dram_tensor(f"xt{s}", [C, cap], bf16, kind="ExternalInput")
        dram[f"w1t{s}"] = nc.dram_tensor(f"w1t{s}", [P, MHP, C], bf16, kind="ExternalInput")
        dram[f"w2t{s}"] = nc.dram_tensor(f"w2t{s}", [P, MHP, C], bf16, kind="ExternalInput")
        dram[f"b1{s}"] = nc.dram_tensor(f"b1{s}", [P, MHP], f32, kind="ExternalInput")
        dram[f"yt{s}"] = nc.dram_tensor(f"yt{s}", [C, cap], bf16, kind="ExternalOutput")

    xr = [dram[f"xt{s}"].rearrange("(ko ki) t -> ki ko t", ki=P) for s in range(NS)]
    yr = [dram[f"yt{s}"].rearrange("(co p) t -> p co t", p=P) for s in range(NS)]

    tiles = [_split_tiles(caps[s], ramp=(s == 0)) for s in range(NS)]
    sched = []
    slot_start = []
    for s in range(NS):
        t0 = 0
        slot_start.append(len(sched))
        for T in tiles[s]:
            sched.append((s, T, t0))
            t0 += T
    # issue slot-k constants two tiles before slot k starts (>=40us of slack)
    const_at = {max(2, slot_start[s] - 2): s for s in range(1, NS)}

    with tile.TileContext(nc) as tc:
        with (
            tc.tile_pool(name="const", bufs=1) as const,
            tc.tile_pool(name="xp", bufs=2) as xp,
            tc.tile_pool(name="gp", bufs=2) as gp,
            tc.tile_pool(name="yp", bufs=6) as yp,
            tc.tile_pool(name="psum", bufs=8, space="PSUM") as psum,
        ):
            w1_sb = [
                const.tile([P, MHP, C], bf16, tag=f"w1{s}", name=f"w1{s}")
                for s in range(NS)
            ]
            w2_sb = [
                const.tile([P, MHP, C], bf16, tag=f"w2{s}", name=f"w2{s}")
                for s in range(NS)
            ]
            b1_sb = [
                const.tile([P, MHP], f32, tag=f"b1{s}", name=f"b1{s}")
                for s in range(NS)
            ]

            def load_consts(s, first_on_scalar=0):
                for m in range(MHP):
                    eng = nc.scalar if m < first_on_scalar else nc.gpsimd
                    eng.dma_start(w1_sb[s][:, m : m + 1, :], dram[f"w1t{s}"][:, m : m + 1, :])
                for ho in range(MHP):
                    nc.gpsimd.dma_start(
                        w2_sb[s][:, ho : ho + 1, :], dram[f"w2t{s}"][:, ho : ho + 1, :]
                    )

            # --- initial loads: x0 on sync; b1+first w1 m-chunks on scalar
            # (fast HW queue); the rest of slot 0 on gpsimd.
            x_tiles = {}
            T0 = tiles[0][0]
            x_tiles[0] = xp.tile([P, KO1, T_TILE], bf16, tag="x", name="x0")
            nc.sync.dma_start(x_tiles[0][:, :, :T0], xr[0][:, :, :T0])
            nc.scalar.dma_start(b1_sb[0][:], dram["b10"][:])
            load_consts(0, first_on_scalar=2)

            for gi, (s, T, t0) in enumerate(sched):
                if gi + 1 < len(sched):
                    ns_, nT, nt0 = sched[gi + 1]
                    x_tiles[gi + 1] = xp.tile(
                        [P, KO1, T_TILE], bf16, tag="x", name=f"x{gi + 1}"
                    )
                    nc.sync.dma_start(
                        x_tiles[gi + 1][:, :, :nT], xr[ns_][:, :, nt0 : nt0 + nT]
                    )
                if gi in const_at:
                    sc = const_at[gi]
                    nc.gpsimd.dma_start(b1_sb[sc][:], dram[f"b1{sc}"][:])
                    load_consts(sc)

                x_sb = x_tiles.pop(gi)
                g_sb = gp.tile([P, MHP, T_TILE], bf16, tag="g", name=f"g{gi}")
                for m in range(MHP):
                    ph = psum.tile([P, T_TILE], f32, tag="ps", name=f"ph{gi}_{m}")
                    for ko in range(KO1):
                        nc.tensor.matmul(
                            ph[:, :T],
                            w1_sb[s][:, m, ko * P : (ko + 1) * P],
                            x_sb[:, ko, :T],
                            start=(ko == 0),
                            stop=(ko == KO1 - 1),
                        )
                    nc.scalar.activation(
                        g_sb[:, m, :T], ph[:, :T], gelu, bias=b1_sb[s][:, m : m + 1]
                    )
                # layer 2, contraction split in two so the first half issues
                # behind the L1 matmuls while the last gelu drains.
                pys = [
                    psum.tile([P, T_TILE], f32, tag="ps", name=f"py{gi}_{co}")
                    for co in range(KO1)
                ]
                for co in range(KO1):
                    for ho in range(MHP // 2):
                        nc.tensor.matmul(
                            pys[co][:, :T],
                            w2_sb[s][:, ho, co * P : (co + 1) * P],
                            g_sb[:, ho, :T],
                            start=(ho == 0),
                            stop=False,
                        )
                for co in range(KO1):
                    for ho in range(MHP // 2, MHP):
                        nc.tensor.matmul(
                            pys[co][:, :T],
                            w2_sb[s][:, ho, co * P : (co + 1) * P],
                            g_sb[:, ho, :T],
                            start=False,
                            stop=(ho == MHP - 1),
                        )
                    y_sb = yp.tile([P, T_TILE], bf16, tag="y", name=f"y{gi}_{co}")
                    nc.vector.tensor_copy(y_sb[:, :T], pys[co][:, :T])
                    # final tile: drain on both HW queues (no more activations
                    # behind the scalar ring to stall)
                    last = gi == len(sched) - 1
                    eng = nc.scalar if (last and co >= KO1 // 2) else nc.sync
                    eng.dma_start(yr[s][:, co, t0 : t0 + T], y_sb[:, :T])
    nc.finalize()
    return nc


def _route(flat_f32: np.ndarray, gate_w: np.ndarray):
    """Router, bit-matching the reference's jax ops (same env/backend)."""
    import jax
    import jax.numpy as jnp

    logits = jnp.asarray(flat_f32) @ jnp.asarray(gate_w).T
    probs = jax.nn.softmax(logits, axis=-1)
    top_p, top_i = jax.lax.top_k(probs, TOP_K)
    weights = top_p / (jnp.sum(top_p, axis=-1, keepdims=True) + 1e-8)
    return np.asarray(top_i), np.asarray(weights)


# results of the last device run, for test harness introspection
last_result = None


def _ensure_ntff_hook():
    """bass_utils' trace path imports antenv.axon_hooks, which the agent
    image's antenv lacks. Build the hook from trn_agent_boot's ctypes
    shim and inject a stand-in module."""
    import sys
    import types

    if "antenv.axon_hooks" in sys.modules:
        return
    try:
        from trn_agent_boot.trn_boot import _ntff_profile_via_ctypes

        hook = _ntff_profile_via_ctypes("/opt/axon/libaxon_pjrt.so")
    except Exception:
        hook = None
    m = types.ModuleType("antenv.axon_hooks")
    m.get_axon_ntff_profile_hook = lambda: hook
    m.set_axon_ntff_profile_hook = lambda h: None
    sys.modules["antenv.axon_hooks"] = m


def _prep_w1(w1e_part: np.ndarray) -> np.ndarray:
    # [HP, C] -> [ki, m, ko*128+j] holding w1[m*128+j, ko*128+ki]
    bf16 = ml_dtypes.bfloat16
    a = w1e_part.reshape(MHP, P, KO1, P)            # [m, j, ko, ki]
    a = a.transpose(3, 0, 2, 1).reshape(P, MHP, C)  # [ki, m, (ko j)]
    return np.ascontiguousarray(a).astype(bf16)


def _prep_w2(w2e_part: np.ndarray) -> np.ndarray:
    # [C, HP] -> [ki, ho, co*128+j] holding w2[co*128+j, ho*128+ki]
    bf16 = ml_dtypes.bfloat16
    a = w2e_part.reshape(KO1, P, MHP, P)            # [co, j, ho, ki]
    a = a.transpose(3, 2, 0, 1).reshape(P, MHP, C)  # [ki, ho, (co j)]
    return np.ascontiguousarray(a).astype(bf16)


def kernel(x, gate_w, w1, b1, w2, b2):
    from concourse.bass_utils import run_bass_kernel_spmd

    x = np.asarray(x)
    B, N, _ = x.shape
    flat = np.ascontiguousarray(x.reshape(-1, C), dtype=np.float32)
    w1 = np.asarray(w1, dtype=np.float32)
    w2 = np.asarray(w2, dtype=np.float32)
    b1 = np.asarray(b1, dtype=np.float32)
    b2 = np.asarray(b2, dtype=np.float32)

    top_i, weights = _route(flat, np.asarray(gate_w, dtype=np.float32))

    idx_e, g_e = [], []
    for e in range(N_EXPERTS):
        rows, cols = np.nonzero(top_i == e)
        idx_e.append(rows)
        g_e.append(weights[rows, cols].astype(np.float32))
    counts = np.array([len(i) for i in idx_e])

    # rank experts by load. Slot s serves experts ranked[s*EPS:(s+1)*EPS];
    # core c computes hidden-part (c // EPS) of expert ranked[s*EPS + c%EPS].
    ranked = np.argsort(-counts, kind="stable")
    caps = tuple(
        int(max(counts[ranked[s * EPS + j]] for j in range(EPS)))
        for s in range(NS)
    )

    nc = _nc_cache.get(caps)
    if nc is None:
        nc = _build_nc(caps)
        _nc_cache[caps] = nc

    bf16 = ml_dtypes.bfloat16

    # per-expert padded x, shared by all cores serving that expert
    xt = {}
    for s in range(NS):
        for j in range(EPS):
            e = int(ranked[s * EPS + j])
            xe = np.zeros((C, caps[s]), dtype=bf16)
            xe[:, : counts[e]] = flat[idx_e[e]].T.astype(bf16)
            xt[e] = xe

    in_maps = []
    for core in range(8):
        j, hpart = core % EPS, core // EPS
        m = {}
        lo, hi = hpart * HP, (hpart + 1) * HP
        for s in range(NS):
            e = int(ranked[s * EPS + j])
            m[f"xt{s}"] = xt[e]
            m[f"w1t{s}"] = _prep_w1(w1[e, lo:hi, :])
            m[f"w2t{s}"] = _prep_w2(w2[e, :, lo:hi])
            m[f"b1{s}"] = np.ascontiguousarray(b1[e, lo:hi].reshape(MHP, P).T)
        in_maps.append(m)

    trace = bool(int(os.environ.get("MOE_TRACE", "0")))
    if trace:
        _ensure_ntff_hook()

    global last_result
    res = run_bass_kernel_spmd(
        nc,
        in_maps,
        core_ids=list(range(8)),
        trace=trace,
    )
    last_result = res

    out = np.zeros((flat.shape[0], C), dtype=np.float32)
    for s in range(NS):
        for j in range(EPS):
            e = int(ranked[s * EPS + j])
            cnt = counts[e]
            y = np.zeros((C, cnt), dtype=np.float32)
            for hpart in range(NS):
                y += res.results[hpart * EPS + j][f"yt{s}"][:, :cnt].astype(np.float32)
            out[idx_e[e]] += g_e[e][:, None] * (y.T + b2[e])
    return out.reshape(B, N, C)
